# revision 11
# baseline (speedup 1.0000x reference)
"""FFN-in-head attention Trainium2 kernel (8 NeuronCores, SPMD).

Sharding: core = (batch b, token-half). Each core runs the q/k/v MLPs for its
1024 tokens (all matmuls bf16, fp32 PSUM accumulate), pairwise-AllGathers
K (channel-major bf16) and V (token-major bf16) between the two cores of a
batch, computes all 16 heads of attention for its 1024 query tokens
(logits transposed [keys, queries]; softmax denominator via a ones-column
appended to V; reciprocal broadcast via gpsimd partition_broadcast), and
finishes with the output projection (bias folded in as a K=1 matmul).

The q-MLP fc2 is interleaved with attention per head-pair, and within each
attention stage the logits matmul+exp pairs alternate with the previous
stage's prob@V matmuls, so the Exp work (scalar engine) hides under tensor
engine work. Weights are host-pre-tiled so DMA lines are >= 1KB contiguous.

PSUM budget (8 banks): shared transient pool "ps" ([P,512], bufs=2) for
fc1 / fc2 / proj = 2 banks; logits pool "lg" ([P,2,512], bufs=2) = 4 banks;
pav pool "pv" ([P,512], bufs=2) = 2 banks.
"""
import sys
sys.path.insert(0, "/opt/trn_rl_repo")
import contextlib
import numpy as np

DIM = 1024
HEADS = 16
HD = 64
HIDDEN = 4096
B = 4
N = 2048
NT = 1024          # tokens per core
P = 128
N_CORES = 8
CH = 2             # token chunks per core in the MLP phase
CHT = NT // CH     # 512 tokens per chunk
HT = HIDDEN // P   # 32 hidden tiles
CT = DIM // P      # 8 channel tiles
KT = N // P        # 16 key tiles
QT = NT // P       # 8 query-token tiles

RG = [[0, 1], [2, 3], [4, 5], [6, 7]]


def build_program(n_cores=N_CORES, with_collectives=True, loop_n=None,
                  sections=("mlp", "attn", "proj"), use_pbcast=False,
                  sim_safe=False):
    import concourse.bass as bass
    import concourse.mybir as mybir
    import concourse.tile as tile
    from concourse import bacc

    fp32 = mybir.dt.float32
    bf16 = mybir.dt.bfloat16
    AF = mybir.ActivationFunctionType
    GELU = AF.Identity if sim_safe else AF.Gelu

    nc = bacc.Bacc("TRN2", target_bir_lowering=False, debug=False,
                   num_devices=n_cores)

    # ---- DRAM I/O ----
    xT = nc.dram_tensor("xT", [DIM, NT], bf16, kind="ExternalInput").ap()
    w1s, b1s, w2s = {}, {}, {}
    for m in ("q", "k", "v"):
        # host pre-tiled: [HT, P, CT*128] so each [P, 2KB] line is contiguous
        w1s[m] = nc.dram_tensor(f"{m}_w1t", [HT, P, DIM], bf16,
                                kind="ExternalInput").ap()
        b1s[m] = nc.dram_tensor(f"{m}_b1r", [P, HT], fp32,
                                kind="ExternalInput").ap()
        if m in ("q", "k"):
            # host pre-tiled: [CT, P, HT*128]
            w2s[m] = nc.dram_tensor(f"{m}_w2t", [CT, P, HIDDEN], bf16,
                                    kind="ExternalInput").ap()
    # v fc2 moving operand: natural [HIDDEN, DIM] layout (2KB lines)
    w2vd = nc.dram_tensor("v_w2", [HIDDEN, DIM], bf16,
                          kind="ExternalInput").ap()
    qb2 = nc.dram_tensor("q_b2r", [P, CT], fp32, kind="ExternalInput").ap()
    kb2 = nc.dram_tensor("k_b2r", [P, CT], fp32, kind="ExternalInput").ap()
    vb2 = nc.dram_tensor("v_b2r", [1, DIM], bf16, kind="ExternalInput").ap()
    pw = nc.dram_tensor("proj_w", [DIM, DIM], bf16, kind="ExternalInput").ap()
    pbr = nc.dram_tensor("proj_br", [1, DIM], bf16, kind="ExternalInput").ap()
    ones_r = nc.dram_tensor("ones_r", [1, P], bf16, kind="ExternalInput").ap()
    out = nc.dram_tensor("out", [NT, DIM], fp32, kind="ExternalOutput").ap()
    if not with_collectives:
        kfull_in = nc.dram_tensor("kfull", [2 * DIM, NT], bf16,
                                  kind="ExternalInput").ap()
        vfull_in = nc.dram_tensor("vfull", [2 * NT, DIM], bf16,
                                  kind="ExternalInput").ap()

    w1_r = {m: w1s[m].rearrange("ht p (kc c) -> p ht kc c", c=P)
            for m in w1s}
    w2_r = {m: w2s[m].rearrange("ct p (kh c) -> p ct kh c", c=P)
            for m in w2s}
    w2v_r = w2vd.rearrange("(kh p) c -> p kh c", p=P)

    def fc1(psp, w1p, m, ch, hT_sb, b1_sb, xT_sb):
        """fc1 + gelu for one 512-token chunk -> hT_sb [P, HT, CHT] bf16."""
        tok = slice(ch * CHT, (ch + 1) * CHT)
        for ht in range(HT):
            w1t = w1p.tile([P, CT, P], bf16, tag="w1t", name="w1t")
            nc.sync.dma_start(w1t[:], w1_r[m][:, ht])
            ps = psp.tile([P, CHT], fp32, tag="ps", name="ps1")
            for kc in range(CT):
                nc.tensor.matmul(
                    ps[:], w1t[:, kc, :], xT_sb[:, kc, tok],
                    start=(kc == 0), stop=(kc == CT - 1))
            nc.scalar.activation(
                hT_sb[:, ht, :], ps[:], GELU, bias=b1_sb[:, ht:ht + 1])

    def fc2_ct(psp, w2p, stg, m, ct, hT_chunks, b2_sb, q_tile, ag_k_in):
        """fc2 channel tile ct (both token chunks) for k or q.

        For q, writes q_tile [P, NT]; for k, stages to DRAM ag_k_in.
        """
        w2t = w2p.tile([P, HT, P], bf16, tag="w2t", name="w2t")
        nc.sync.dma_start(w2t[:], w2_r[m][:, ct])
        for ch in range(CH):
            tok = slice(ch * CHT, (ch + 1) * CHT)
            ps = psp.tile([P, CHT], fp32, tag="ps", name="ps2")
            for kh in range(HT):
                nc.tensor.matmul(
                    ps[:], w2t[:, kh, :], hT_chunks[ch][:, kh, :],
                    start=(kh == 0), stop=(kh == HT - 1),
                    skip_group_check=True)
            if m == "q":
                nc.scalar.activation(
                    q_tile[:, tok], ps[:], AF.Identity,
                    bias=b2_sb[:, ct:ct + 1])
            else:
                kst = stg.tile([P, CHT], bf16, tag="kst", name="kst")
                nc.scalar.activation(
                    kst[:], ps[:], AF.Identity, bias=b2_sb[:, ct:ct + 1])
                nc.sync.dma_start(ag_k_in[ct * P:(ct + 1) * P, tok], kst[:])

    def fc2_v(psp, stg, ch, hT_sb, w2v_tiles, ones_sb, vb2_sb, ag_v_in):
        """fc2 token-major for v, one chunk: y[tok, c] = h.T @ w2 + b2.

        Two psum tiles at a time (token-tile pairs) to stay within the
        shared 2-buf "ps" pool.
        """
        for nt in range(2):
            cs = slice(nt * 512, (nt + 1) * 512)
            for tp in range(2):          # token-tile pairs (tt = 2*tp+j)
                pss = [psp.tile([P, 512], fp32, tag="ps", name=f"psv{j}")
                       for j in range(2)]
                for kh in range(HT):
                    w2vt = w2v_tiles[kh]
                    for j in range(2):
                        tt = 2 * tp + j
                        nc.tensor.matmul(
                            pss[j][:],
                            hT_sb[:, kh, tt * P:(tt + 1) * P],
                            w2vt[:, cs], start=(kh == 0), stop=False,
                            skip_group_check=True)
                for j in range(2):
                    tt = 2 * tp + j
                    nc.tensor.matmul(
                        pss[j][:], ones_sb[:, :], vb2_sb[:, cs],
                        start=False, stop=True, skip_group_check=True)
                    vst = stg.tile([P, 512], bf16, tag="vst", name="vst")
                    nc.vector.tensor_copy(vst[:], pss[j][:])
                    nc.sync.dma_start(
                        ag_v_in[ch * CHT + tt * P:ch * CHT + (tt + 1) * P,
                                cs], vst[:])

    def proj_phase(psp, ones_sb, pbr_sb, aoT_sb):
        pw_r = pw.rearrange("(kc p) c -> p kc c", p=P)
        with nc.tc.tile_pool(name="proj", bufs=1) as pjp:
            pwt = {}
            for kc in range(CT):
                t = pjp.tile([P, DIM], bf16, tag=f"pwt{kc}", name="pwt")
                nc.sync.dma_start(t[:], pw_r[:, kc, :])
                pwt[kc] = t
            for tt in range(QT):
                for nt in range(2):
                    cs = slice(nt * 512, (nt + 1) * 512)
                    ps = psp.tile([P, 512], fp32, tag="ps", name="psp")
                    for kc in range(CT):
                        nc.tensor.matmul(
                            ps[:], aoT_sb[:, kc, tt * P:(tt + 1) * P],
                            pwt[kc][:, cs], start=(kc == 0), stop=False,
                            skip_group_check=True)
                    nc.tensor.matmul(
                        ps[:], ones_sb[:, :], pbr_sb[:, cs],
                        start=False, stop=True, skip_group_check=True)
                    ot = pjp.tile([P, 512], fp32, tag="ot", bufs=3, name="ot")
                    nc.vector.tensor_copy(ot[:], ps[:])
                    nc.sync.dma_start(out[tt * P:(tt + 1) * P, cs], ot[:])

    # ---------------- attention ----------------
    # stage s = (hp, nt): 512 query tokens of head-pair hp.
    # Per stage: 16 logits psum pairs (2 ktl each) -> exp -> probs tiles;
    # prob@V accumulation of the PREVIOUS stage's probs is interleaved two
    # matmuls per logits pair so the tensor engine never waits on Exp.
    def make_attn(pslg, psv, atp, kvp, dnp, dnd, q_tiles, dram_t, aoT_sb):
        (ag_k_in, ag_v_in, ag_k_out, ag_v_out) = dram_t
        k_r = ag_k_out.rearrange("(half ct p) t -> p ct half t",
                                 half=2, ct=CT, p=P)
        v_r = ag_v_out.rearrange("(ktl p) c -> p ktl c", p=P)
        state = {}
        scale = float(HD) ** -0.5

        def emit_kv(hp):
            k2 = kvp.tile([P, 2, NT], bf16, tag="k2", name="k2")
            nc.sync.dma_start(k2[:], k_r[:, hp])
            vA = kvp.tile([P, KT, P], bf16, tag="vA", name="vA")
            vB = kvp.tile([P, KT, P], bf16, tag="vB", name="vB")
            cA, cB = 2 * hp * HD, (2 * hp + 1) * HD
            nc.gpsimd.memset(vA[:, :, HD:], 0.0)
            nc.gpsimd.memset(vA[:, :, HD:HD + 1], 1.0)
            nc.sync.dma_start(vA[:, :, 0:HD], v_r[:, :, cA:cA + HD])
            nc.gpsimd.memset(vB[:, :, 0:HD], 0.0)
            nc.gpsimd.memset(vB[:, :, 0:1], 1.0)
            nc.sync.dma_start(vB[:, :, HD:], v_r[:, :, cB:cB + HD])
            state[("kv", hp)] = (k2, vA, vB)

        def pav_drain(head, pav, hp, nt, rd):
            """Reciprocal of the softmax denominator and scale-out."""
            qs = slice(nt * 512, (nt + 1) * 512)
            dn = dnp.tile([1, 512], fp32, tag=f"dn{head}", name="dn",
                          bufs=1)
            drow = HD if head == 0 else 0
            nc.vector.tensor_copy(dn[0:1, :], pav[drow:drow + 1, :])
            rec = dnp.tile([1, 512], fp32, tag=f"rec{head}", name="rec",
                           bufs=1)
            nc.vector.reciprocal(rec[0:1, :], dn[0:1, :])
            rows = slice(0, HD) if head == 0 else slice(HD, P)
            if use_pbcast:
                nc.gpsimd.partition_broadcast(rd[rows, :], rec[0:1, :])
            else:
                drec = dnd.tile([1, 512], fp32, tag=f"drec{head}",
                                name="drec")
                nc.sync.dma_start(drec[:], rec[0:1, :])
                nc.sync.dma_start(rd[rows, :],
                                  drec[0:1, :].broadcast_to([HD, 512]))
            nc.vector.tensor_mul(
                aoT_sb[rows, hp, qs], pav[rows, :], rd[rows, :])

        def emit_stage(s, prev):
            """Emit logits+exp for stage s, interleaving pav of `prev`."""
            if s is not None:
                hp, nt = s
                k2, vA, vB = state[("kv", hp)]
                qs = slice(nt * 512, (nt + 1) * 512)
                atA = atp.tile([P, KT, 512], bf16, tag="atA", name="atA")
                atB = atp.tile([P, KT, 512], bf16, tag="atB", name="atB")
                state[("at", hp, nt)] = (atA, atB)
            if prev is not None:
                php, pnt = prev
                pk2, pvA_t, pvB_t = state[("kv", php)]
                patA, patB = state.pop(("at", php, pnt))
                pav_tiles = {}
                rd = dnp.tile([P, 512], fp32, tag="rd", name="rd")
            # 16 interleave steps: logits pair + 2 pav matmuls each
            for idx in range(16):
                if s is not None:
                    head, kp = divmod(idx, 8)
                    at = atA if head == 0 else atB
                    rows = slice(0, HD) if head == 0 else slice(HD, P)
                    ps = pslg.tile([P, 2, 512], fp32, tag="lg", name="pslg")
                    for j in range(2):
                        ktl = 2 * kp + j
                        half, col = ktl // CT, (ktl % CT) * P
                        nc.tensor.matmul(
                            ps[:, j, :], k2[rows, half, col:col + P],
                            q_tiles[hp][rows, qs], start=True, stop=True,
                            skip_group_check=True)
                    nc.scalar.activation(
                        at[:, 2 * kp:2 * kp + 2, :], ps[:], AF.Exp,
                        scale=scale)
                if prev is not None:
                    phead, pkp = divmod(idx, 8)
                    pat = patA if phead == 0 else patB
                    pvv = pvA_t if phead == 0 else pvB_t
                    if pkp == 0:
                        pav_tiles[phead] = psv.tile([P, 512], fp32, tag="pv",
                                                    name="pav")
                    for j in range(2):
                        ktl = 2 * pkp + j
                        nc.tensor.matmul(
                            pav_tiles[phead][:], pvv[:, ktl, :],
                            pat[:, ktl, :], start=(ktl == 0),
                            stop=(ktl == KT - 1), skip_group_check=True)
                    if pkp == 7:
                        pav_drain(phead, pav_tiles[phead], php, pnt, rd)

        return emit_kv, emit_stage

    with tile.TileContext(nc) as tc:
        nc.tc = tc
        loop_ctx = tc.For_i(0, loop_n, 1) if loop_n else contextlib.nullcontext()
        with loop_ctx, \
             tc.tile_pool(name="outer", bufs=1) as outer, \
             tc.tile_pool(name="ps", bufs=2, space="PSUM") as psp, \
             tc.tile_pool(name="pslg", bufs=2, space="PSUM") as pslg, \
             tc.tile_pool(name="psv", bufs=2, space="PSUM") as psv, \
             tc.tile_pool(name="dram", bufs=1, space="DRAM") as dram:

            qT_tiles = {}
            ones_sb = outer.tile([1, P], bf16, tag="ones", name="oness")
            nc.sync.dma_start(ones_sb[:], ones_r[:])
            vb2_sb = outer.tile([1, DIM], bf16, tag="vb2", name="vb2s")
            nc.sync.dma_start(vb2_sb[:], vb2[:])
            pbr_sb = outer.tile([1, DIM], bf16, tag="pbr", name="pbrs")
            nc.sync.dma_start(pbr_sb[:], pbr[:])
            aoT_sb = outer.tile([P, CT, NT], bf16, tag="aoT", name="aoTs")
            qb2_sb = outer.tile([P, CT], fp32, tag="b2q", name="b2qs")
            nc.sync.dma_start(qb2_sb[:], qb2[:])

            ag_k_in = dram.tile([DIM, NT], bf16, tag="agki", name="agki")
            ag_v_in = dram.tile([NT, DIM], bf16, tag="agvi", name="agvi")
            if with_collectives:
                ag_k_out = dram.tile([2 * DIM, NT], bf16, tag="agko",
                                     name="agko")
                ag_v_out = dram.tile([2 * NT, DIM], bf16, tag="agvo",
                                     name="agvo")
            else:
                ag_k_out, ag_v_out = kfull_in, vfull_in
            dram_t = (ag_k_in, ag_v_in, ag_k_out, ag_v_out)

            import concourse.mybir as mybir_

            if "mlp" in sections:
                with tc.tile_pool(name="hTp", bufs=2) as hTp, \
                     tc.tile_pool(name="w2p", bufs=2) as w2p, \
                     tc.tile_pool(name="qTp", bufs=3) as qTp:
                    hT_chunks = {}
                    with tc.tile_pool(name="xTp", bufs=1) as xTp, \
                         tc.tile_pool(name="w1p", bufs=3) as w1p, \
                         tc.tile_pool(name="mlp", bufs=1) as mlp, \
                         tc.tile_pool(name="stg", bufs=3) as stg:
                        xT_sb = xTp.tile([P, CT, NT], bf16, tag="xT",
                                         name="xTs")
                        nc.sync.dma_start(
                            xT_sb[:],
                            xT.rearrange("(ct p) t -> p ct t", p=P))
                        # ---- k and v MLPs ----
                        for m in ("k", "v"):
                            b1_sb = mlp.tile([P, HT], fp32, tag=f"b1{m}",
                                             name="b1")
                            nc.sync.dma_start(b1_sb[:], b1s[m][:])
                            if m == "k":
                                b2_sb = mlp.tile([P, CT], fp32, tag="b2k",
                                                 name="b2")
                                nc.sync.dma_start(b2_sb[:], kb2[:])
                                kh_chunks = {}
                                for ch in range(CH):
                                    hT_sb = hTp.tile([P, HT, CHT], bf16,
                                                     tag="hT", name="hT")
                                    fc1(psp, w1p, m, ch, hT_sb, b1_sb, xT_sb)
                                    kh_chunks[ch] = hT_sb
                                for ct in range(CT):
                                    fc2_ct(psp, w2p, stg, m, ct, kh_chunks,
                                           b2_sb, None, ag_k_in)
                            else:
                                w2v_tiles = {}
                                with tc.tile_pool(name="w2vp", bufs=1) as w2vp:
                                    for kh in range(HT):
                                        t = w2vp.tile([P, DIM], bf16,
                                                      tag=f"w2v{kh}",
                                                      name="w2vt")
                                        nc.sync.dma_start(t[:],
                                                          w2v_r[:, kh, :])
                                        w2v_tiles[kh] = t
                                    for ch in range(CH):
                                        hT_sb = hTp.tile([P, HT, CHT], bf16,
                                                         tag="hT", name="hT")
                                        fc1(psp, w1p, m, ch, hT_sb, b1_sb,
                                            xT_sb)
                                        fc2_v(psp, stg, ch, hT_sb, w2v_tiles,
                                              ones_sb, vb2_sb, ag_v_in)
                            if with_collectives and m == "k":
                                nc.gpsimd.collective_compute(
                                    "AllGather", mybir_.AluOpType.bypass,
                                    replica_groups=RG,
                                    ins=[ag_k_in.opt()],
                                    outs=[ag_k_out.opt()])
                            elif with_collectives and m == "v":
                                nc.gpsimd.collective_compute(
                                    "AllGather", mybir_.AluOpType.bypass,
                                    replica_groups=RG,
                                    ins=[ag_v_in.opt()],
                                    outs=[ag_v_out.opt()])

                        # ---- q fc1 (both chunks) ----
                        b1_sb = mlp.tile([P, HT], fp32, tag="b1q", name="b1")
                        nc.sync.dma_start(b1_sb[:], b1s["q"][:])
                        for ch in range(CH):
                            hT_sb = hTp.tile([P, HT, CHT], bf16, tag="hT",
                                             name="hT")
                            fc1(psp, w1p, "q", ch, hT_sb, b1_sb, xT_sb)
                            hT_chunks[ch] = hT_sb

                    def q_fc2(ct):
                        t = qTp.tile([P, NT], bf16, tag="qT", name="qTt")
                        qT_tiles[ct] = t
                        fc2_ct(psp, w2p, None, "q", ct, hT_chunks, qb2_sb,
                               t, ag_k_in)

                    # ---- q fc2 interleaved with attention ----
                    if "attn" in sections:
                        with tc.tile_pool(name="attp", bufs=2) as atp, \
                             tc.tile_pool(name="kvp", bufs=2) as kvp, \
                             tc.tile_pool(name="dnp", bufs=2) as dnp:
                            emit_kv, emit_stage = make_attn(
                                pslg, psv, atp, kvp, dnp, dram, qT_tiles,
                                dram_t, aoT_sb)
                            q_fc2(0)
                            emit_kv(0)
                            q_fc2(1)
                            stages = [(hp, nt) for hp in range(CT)
                                      for nt in range(2)]
                            prev = None
                            for (hp, nt) in stages:
                                if nt == 1 and hp + 1 < CT:
                                    emit_kv(hp + 1)
                                emit_stage((hp, nt), prev)
                                if nt == 1 and hp + 2 < CT:
                                    q_fc2(hp + 2)
                                prev = (hp, nt)
                            emit_stage(None, prev)
                    else:
                        for ct in range(CT):
                            q_fc2(ct)

            if "proj" in sections and "attn" in sections:
                proj_phase(psp, ones_sb, pbr_sb, aoT_sb)
            else:
                ot0 = outer.tile([P, 512], fp32, tag="ot0", name="ot0")
                if "attn" in sections:
                    nc.vector.tensor_copy(ot0[:], aoT_sb[:, 0, 0:512])
                elif "mlp" in sections:
                    nc.vector.tensor_copy(ot0[:], qT_tiles[0][:, 0:512])
                else:
                    nc.vector.memset(ot0[:], 0.0)
                nc.sync.dma_start(out[0:P, 0:512], ot0[:])

    nc.compile()
    return nc


_CACHE = {}


def _get_runner(loop_n=None, **bkw):
    import os
    use_pbcast = os.environ.get("KERNEL_PBCAST", "0") == "1"
    key = ("runner", loop_n, use_pbcast, tuple(sorted(bkw.items())))
    if key in _CACHE:
        return _CACHE[key]
    import jax
    from jax.sharding import Mesh, PartitionSpec
    from jax.experimental.shard_map import shard_map
    from concourse import mybir
    from concourse.bass2jax import (_bass_exec_p, partition_id_tensor,
                                    install_neuronx_cc_hook)

    nc = build_program(loop_n=loop_n, use_pbcast=use_pbcast, **bkw)
    install_neuronx_cc_hook()
    partition_name = nc.partition_id_tensor.name if nc.partition_id_tensor else None
    in_names, out_names, out_avals = [], [], []
    for alloc in nc.m.functions[0].allocations:
        if not isinstance(alloc, mybir.MemoryLocationSet):
            continue
        name = alloc.memorylocations[0].name
        if alloc.kind == "ExternalInput":
            if name != partition_name:
                in_names.append(name)
        elif alloc.kind == "ExternalOutput":
            out_names.append(name)
            out_avals.append(jax.core.ShapedArray(
                tuple(alloc.tensor_shape), mybir.dt.np(alloc.dtype)))
    n_params, n_outs = len(in_names), len(out_avals)
    all_in_names = list(in_names) + list(out_names)
    if partition_name is not None:
        all_in_names.append(partition_name)
    donate = tuple(range(n_params, n_params + n_outs))

    def _body(*args):
        operands = list(args)
        if partition_name is not None:
            operands.append(partition_id_tensor())
        outs = _bass_exec_p.bind(
            *operands, out_avals=tuple(out_avals), in_names=tuple(all_in_names),
            out_names=tuple(out_names), lowering_input_output_aliases=(),
            sim_require_finite=True, sim_require_nnan=True, nc=nc)
        return tuple(outs)

    devices = jax.devices()[:N_CORES]
    mesh = Mesh(np.asarray(devices), ("core",))
    in_specs = (PartitionSpec("core"),) * (n_params + n_outs)
    out_specs = (PartitionSpec("core"),) * n_outs
    fn = jax.jit(
        shard_map(_body, mesh=mesh, in_specs=in_specs, out_specs=out_specs,
                  check_rep=False),
        donate_argnums=donate, keep_unused=True)
    runner = {"fn": fn, "in_names": in_names, "out_names": out_names,
              "out_avals": out_avals, "mesh": mesh}
    _CACHE[key] = runner
    return runner


def make_in_maps(x, q_w1, q_b1, q_w2, q_b2, k_w1, k_b1, k_w2, k_b2,
                 v_w1, v_b1, v_w2, v_b2, proj_w, proj_b):
    import ml_dtypes
    f32 = np.float32
    bf = ml_dtypes.bfloat16
    x = np.asarray(x, f32)

    def tile_w1(w):  # [DIM, HIDDEN] -> [HT, P, DIM]
        w = np.asarray(w, f32).astype(bf)
        # [ht][p][kc*128+c] = w1[kc*128+p, ht*128+c]
        return np.ascontiguousarray(
            w.reshape(CT, P, HT, P).transpose(2, 1, 0, 3).reshape(HT, P, DIM))

    def tile_w2kq(w):  # [HIDDEN, DIM] -> [CT, P, HIDDEN]
        w = np.asarray(w, f32).astype(bf)
        # [ct][p][kh*128+c] = w2[kh*128+p, ct*128+c]
        return np.ascontiguousarray(
            w.reshape(HT, P, CT, P).transpose(2, 1, 0, 3).reshape(
                CT, P, HIDDEN))

    shared = {
        "q_w1t": tile_w1(q_w1),
        "k_w1t": tile_w1(k_w1),
        "v_w1t": tile_w1(v_w1),
        "q_w2t": tile_w2kq(q_w2),
        "k_w2t": tile_w2kq(k_w2),
        "v_w2": np.ascontiguousarray(np.asarray(v_w2, f32).astype(bf)),
        "q_b1r": np.ascontiguousarray(np.asarray(q_b1, f32).reshape(HT, P).T),
        "k_b1r": np.ascontiguousarray(np.asarray(k_b1, f32).reshape(HT, P).T),
        "v_b1r": np.ascontiguousarray(np.asarray(v_b1, f32).reshape(HT, P).T),
        "q_b2r": np.ascontiguousarray(np.asarray(q_b2, f32).reshape(CT, P).T),
        "k_b2r": np.ascontiguousarray(np.asarray(k_b2, f32).reshape(CT, P).T),
        "v_b2r": np.ascontiguousarray(
            np.asarray(v_b2, f32).reshape(1, DIM).astype(bf)),
        "proj_w": np.ascontiguousarray(np.asarray(proj_w, f32).astype(bf)),
        "proj_br": np.ascontiguousarray(
            np.asarray(proj_b, f32).reshape(1, DIM).astype(bf)),
        "ones_r": np.ones((1, P), bf),
    }
    in_maps = []
    for c in range(N_CORES):
        b, half = c // 2, c % 2
        xT_c = np.ascontiguousarray(
            x[b, half * NT:(half + 1) * NT, :].T.astype(bf))
        in_maps.append({"xT": xT_c, **shared})
    return in_maps


def run_in_maps(in_maps, loop_n=None, **bkw):
    import jax
    from jax.sharding import NamedSharding, PartitionSpec
    r = _get_runner(loop_n=loop_n, **bkw)
    shard = NamedSharding(r["mesh"], PartitionSpec("core"))
    concat_in = [
        np.concatenate([np.asarray(in_maps[c][name]) for c in range(N_CORES)],
                       axis=0)
        for name in r["in_names"]
    ]
    dev_in = [jax.device_put(a, shard) for a in concat_in]
    concat_zeros = [
        np.zeros((N_CORES * av.shape[0], *av.shape[1:]), av.dtype)
        for av in r["out_avals"]
    ]
    out_arrs = r["fn"](*dev_in, *concat_zeros)
    out_arrs = [np.asarray(o) for o in out_arrs]
    return [
        {name: out_arrs[i].reshape(N_CORES, *r["out_avals"][i].shape)[c]
         for i, name in enumerate(r["out_names"])}
        for c in range(N_CORES)
    ]


def kernel(**inputs):
    in_maps = make_in_maps(**inputs)
    results = run_in_maps(in_maps)
    out = np.empty((B, N, DIM), np.float32)
    for c in range(N_CORES):
        b, half = c // 2, c % 2
        out[b, half * NT:(half + 1) * NT, :] = results[c]["out"]
    return out
